# revision 25
# baseline (speedup 1.0000x reference)
"""Trainium2 Bass kernel for nn_ConstrainModule (gnn_message_passing).

Computes, for full inputs:
    A[c,s]   = sum_{n,h,w} seg[n,c,s,h,w] * det[n,c,h,w]
    denom[c] = sum_{n,h,w} det[n,c,h,w]
    p_area   = A / denom[:,None]
    w[j]     = sum over edges (i,j) of p_area[j,i]
    probs    = det_class_probs @ w
    loss     = mean(-clip(log(probs), -100))

Sharding: data-parallel over N_obj (1024 -> 128 per core, 8 cores).

Device strategy per core (n=128 objects on the SBUF partition dim):
  - inputs are packed per det class c as raw bytes: det mask in bf16
    (1568 B) followed by the 4 seg masks in fp8e4m3 (3136 B, stochastic
    rounding on host so the quantizer is unbiased), one ~0.6MB DMA per
    class, alternating between the two HWDGE rings; on device the
    regions are bitcast back.
  - TensorE contracts n: for each c, 7 accumulating matmuls with
    lhsT = det hw-chunk (112 cols, bf16) and rhs = seg (s, hw-chunk)
    (448 cols, fp8) produce psum[g, s*112+g'] cross products in fp32;
    the g==g' diagonal holds sum_n det[n,c,hw_g] * seg[n,c,s,hw_g'].
  - one fused VectorE scalar_tensor_tensor per class multiplies the
    psum block by an edge-weighted diag mask (mask[g, s*112+g'] =
    E[c,s]*(g==g'), built on host from edge_i/edge_j at call time) and
    free-dim-accumulates, yielding numer[c] = sum_s E[c,s]*A[c,s]
    partials directly; denom partials on the ScalarE via
    activation(Copy, accum_out=...), one per c.
  - a short burst of dummy matmuls at kernel start trips the PE HAM
    clock gate during the initial DMA wait so real matmuls run at
    2.4 GHz.
  - host sums the tiny per-core partials and computes the scalar loss.

Precision: stochastic rounding makes the fp8 quantizer unbiased, so the
~800K-term fp32 reductions average the ~2% per-element seg noise down
to ~3e-5 relative; det stays bf16 (~1e-5).

Self-contained: hardcodes all shapes; reads no sibling files.
"""

import numpy as np
import ml_dtypes

import concourse.bacc as bacc
import concourse.mybir as mybir
import concourse.tile as tile
from concourse.bass_utils import run_bass_kernel_spmd

N_CORES = 8
N_OBJ, C_DET, C_SEG, H, W = 1024, 8, 4, 28, 28
HW = H * W                 # 784
NS = N_OBJ // N_CORES      # 128 objects per core -> partition dim
G = 112                    # hw chunk size (lhs free dim); 784 = 7 * 112
KCH = HW // G              # 7 accumulating matmuls per class

DET_B = HW * 2             # 1568 bytes of bf16 det per class
SEG_B = C_SEG * HW         # 3136 bytes of fp8 seg per class
ROW_B = DET_B + SEG_B      # 4704 bytes per (n, c)

F32 = mybir.dt.float32
BF16 = mybir.dt.bfloat16
FP8 = mybir.dt.float8e4
NP_FP8 = ml_dtypes.float8_e4m3
U8 = mybir.dt.uint8

X_BUFS = 4
PSUM_BUFS = 4
WARMUP_MMS = 8

_program = None


def _build_program():
    nc = bacc.Bacc(
        "TRN2", target_bir_lowering=False, debug=False, num_devices=N_CORES
    )
    x_d = nc.dram_tensor("x", [C_DET, NS, ROW_B], U8, kind="ExternalInput")
    # per-class edge-weighted diag masks: mask[g, c, s*G+g'] = E[c,s]*(g==g')
    mask_d = nc.dram_tensor(
        "mask", [G, C_DET, C_SEG * G], BF16, kind="ExternalInput"
    )
    numer_d = nc.dram_tensor("numer", [G, C_DET], F32, kind="ExternalOutput")
    dsum_d = nc.dram_tensor("dsum", [NS, C_DET], F32, kind="ExternalOutput")

    with tile.TileContext(nc) as tc:
        with (
            tc.tile_pool(name="x", bufs=X_BUFS) as x_pool,
            tc.tile_pool(name="res", bufs=1) as res_pool,
            tc.tile_pool(name="psum", bufs=PSUM_BUFS, space="PSUM") as psum_pool,
        ):
            # PE warmup: dense dummy matmuls (zeroed operands) to flip the
            # HAM clock gate to 2.4 GHz while the first input DMA lands.
            warm_t = res_pool.tile([NS, 512], FP8)
            nc.gpsimd.memset(warm_t[:], 0.0)
            warm_ps = psum_pool.tile([8, 512], F32)
            for _ in range(WARMUP_MMS):
                nc.tensor.matmul(
                    warm_ps[:], warm_t[:, :8], warm_t[:, :512],
                    start=True, stop=True,
                )

            mask_t = res_pool.tile([G, C_DET, C_SEG * G], BF16)
            nc.scalar.dma_start(out=mask_t[:], in_=mask_d[:])
            numer_t = res_pool.tile([G, C_DET], F32)
            dsum_t = res_pool.tile([NS, C_DET], F32)
            scratch = res_pool.tile([G, C_SEG * G], F32)
            act_scratch = res_pool.tile([NS, HW], BF16)

            for c in range(C_DET):
                x_t = x_pool.tile([NS, ROW_B], U8)
                nc.sync.dma_start(out=x_t[:], in_=x_d[c])
                det_v = x_t[:, 0:DET_B].bitcast(BF16)               # [NS, 784]
                seg_v = x_t[:, DET_B:ROW_B].bitcast(FP8).rearrange(
                    "p (s hw) -> p s hw", s=C_SEG
                )                                                   # [NS, 4, 784]
                nc.scalar.activation(
                    out=act_scratch[:],
                    in_=det_v,
                    func=mybir.ActivationFunctionType.Copy,
                    accum_out=dsum_t[:, c : c + 1],
                )
                psum_t = psum_pool.tile([G, C_SEG * G], F32)
                for k in range(KCH):
                    nc.tensor.matmul(
                        psum_t[:],
                        det_v[:, k * G : (k + 1) * G],
                        seg_v[:, :, k * G : (k + 1) * G],
                        start=(k == 0),
                        stop=(k == KCH - 1),
                    )
                nc.vector.scalar_tensor_tensor(
                    out=scratch[:],
                    in0=psum_t[:],
                    scalar=0.0,
                    in1=mask_t[:, c, :],
                    op0=mybir.AluOpType.bypass,
                    op1=mybir.AluOpType.mult,
                    accum_out=numer_t[:, c : c + 1],
                )
            nc.sync.dma_start(out=numer_d[:], in_=numer_t[:])
            nc.sync.dma_start(out=dsum_d[:], in_=dsum_t[:])

    nc.compile()
    return nc


def _get_program():
    global _program
    if _program is None:
        _program = _build_program()
    return _program


def _sr_fp8(v, rng):
    """Exact stochastic rounding to fp8e4m3: E[q(v)] = v.

    For non-negative v below fp8 max, the e4m3 bit patterns are monotone,
    so the two neighbors of v are byte-adjacent.
    """
    q0 = v.astype(NP_FP8)
    f0 = q0.astype(np.float32)
    b = q0.view(np.uint8)
    lo_b = np.where(f0 <= v, b, b - 1).astype(np.uint8)
    hi_b = lo_b + 1
    lo = lo_b.view(NP_FP8).astype(np.float32)
    hi = hi_b.view(NP_FP8).astype(np.float32)
    p = (v - lo) / (hi - lo)
    u = rng.random(v.shape, dtype=np.float32)
    out_b = np.where(u < p, hi_b, lo_b).astype(np.uint8)
    # exactly-representable values keep their encoding
    out_b = np.where(f0 == v, b, out_b)
    return out_b.view(NP_FP8)


def _pack_inputs(det_mask_probs, seg_mask_probs):
    """[N,8,28,28] f32 + [N,8,4,28,28] f32 -> [cores, C_DET, NS, ROW_B] u8."""
    det = np.asarray(det_mask_probs, dtype=np.float32).reshape(
        N_CORES, NS, C_DET, HW
    )
    seg = np.asarray(seg_mask_probs, dtype=np.float32).reshape(
        N_CORES, NS, C_DET, C_SEG * HW
    )
    rng = np.random.default_rng(12345)
    det_b = det.astype(ml_dtypes.bfloat16).view(np.uint8)   # [.., C_DET, 1568]
    seg_b = _sr_fp8(seg, rng).view(np.uint8)                # [.., C_DET, 3136]
    packed = np.concatenate([det_b, seg_b], axis=3)         # [8, NS, C_DET, 4704]
    packed = packed.transpose(0, 2, 1, 3)                   # [8, C_DET, NS, 4704]
    return np.ascontiguousarray(packed)


def _edge_mask(edge_i, edge_j):
    """mask[g, c, s*G+g'] = E[c,s] * (g == g')."""
    E = np.zeros((C_DET, C_SEG), dtype=np.float32)
    np.add.at(E, (np.asarray(edge_j), np.asarray(edge_i)), 1.0)
    eye = np.eye(G, dtype=np.float32)
    blk = np.einsum("cs,gh->gcsh", E, eye)      # [G, C_DET, C_SEG, G]
    return np.ascontiguousarray(
        blk.reshape(G, C_DET, C_SEG * G).astype(ml_dtypes.bfloat16)
    )                                           # E counts <= 256: exact in bf16


def _run_device(det_mask_probs, seg_mask_probs, edge_i, edge_j, trace=False):
    """Run the per-core reduction on all 8 cores; return (numer, denom, res)."""
    nc = _get_program()
    x = _pack_inputs(det_mask_probs, seg_mask_probs)
    mask = _edge_mask(edge_i, edge_j)

    in_maps = [{"x": x[r], "mask": mask} for r in range(N_CORES)]
    res = run_bass_kernel_spmd(nc, in_maps, list(range(N_CORES)), trace=trace)

    numer = np.zeros((C_DET,), dtype=np.float64)
    denom = np.zeros((C_DET,), dtype=np.float64)
    for r in range(N_CORES):
        numer += res.results[r]["numer"].sum(axis=0)
        denom += res.results[r]["dsum"].sum(axis=0)
    return numer, denom, res


def _finish(det_class_probs, numer, denom):
    w = numer / denom  # (C_DET,)
    probs = np.asarray(det_class_probs, dtype=np.float64) @ w  # (N_OBJ,)
    bce = (-np.clip(np.log(probs), -100.0, None)).mean()
    return np.asarray(bce, dtype=np.float32)


def kernel(det_class_probs, det_mask_probs, seg_mask_probs, edge_i, edge_j):
    numer, denom, _ = _run_device(
        det_mask_probs, seg_mask_probs, edge_i, edge_j, trace=False
    )
    return _finish(det_class_probs, numer, denom)


# revision 29
# speedup vs baseline: 1.0640x; 1.0640x over previous
"""Trainium2 Bass kernel for nn_ConstrainModule (gnn_message_passing).

Computes, for full inputs:
    A[c,s]   = sum_{n,h,w} seg[n,c,s,h,w] * det[n,c,h,w]
    denom[c] = sum_{n,h,w} det[n,c,h,w]
    p_area   = A / denom[:,None]
    w[j]     = sum over edges (i,j) of p_area[j,i]
    probs    = det_class_probs @ w
    loss     = mean(-clip(log(probs), -100))

Sharding: data-parallel over N_obj (1024 -> 128 per core, 8 cores).

Device strategy per core (n=128 objects on the SBUF partition dim):
  - inputs are packed per det class c as raw bytes: det mask in bf16
    (1568 B) followed by the 4 seg masks in fp8e4m3 (3136 B, stochastic
    rounding on host so the quantizer is unbiased), one ~0.6MB DMA per
    class, alternating between the two HWDGE rings; on device the
    regions are bitcast back.
  - TensorE contracts n: for each c, 7 accumulating matmuls with
    lhsT = det hw-chunk (112 cols, bf16) and rhs = seg (s, hw-chunk)
    (448 cols, fp8) produce psum[g, s*112+g'] cross products in fp32;
    the g==g' diagonal holds sum_n det[n,c,hw_g] * seg[n,c,s,hw_g'].
  - one fused VectorE scalar_tensor_tensor per class multiplies the
    psum block by an edge-weighted diag mask (mask[g, s*112+g'] =
    E[c,s]*(g==g'), built on host from edge_i/edge_j at call time) and
    free-dim-accumulates, yielding numer[c] = sum_s E[c,s]*A[c,s]
    partials directly; denom partials on the ScalarE via
    activation(Copy, accum_out=...), one per c.
  - a short burst of dummy matmuls at kernel start trips the PE HAM
    clock gate during the initial DMA wait so real matmuls run at
    2.4 GHz.
  - host sums the tiny per-core partials and computes the scalar loss.

Precision: stochastic rounding makes the fp8 quantizer unbiased, so the
~800K-term fp32 reductions average the ~2% per-element seg noise down
to ~3e-5 relative; det stays bf16 (~1e-5).

Self-contained: hardcodes all shapes; reads no sibling files.
"""

import numpy as np
import ml_dtypes

import concourse.bacc as bacc
import concourse.mybir as mybir
import concourse.tile as tile
from concourse.bass_utils import run_bass_kernel_spmd

N_CORES = 8
N_OBJ, C_DET, C_SEG, H, W = 1024, 8, 4, 28, 28
HW = H * W                 # 784
NS = N_OBJ // N_CORES      # 128 objects per core -> partition dim
G = 112                    # hw chunk size (lhs free dim); 784 = 7 * 112
KCH = HW // G              # 7 accumulating matmuls per class
CPC = 2                    # det classes per DMA chunk (~1.2MB chunks)
NCH = C_DET // CPC         # 4 chunks

DET_B = HW * 2             # 1568 bytes of bf16 det per class
SEG_B = C_SEG * HW         # 3136 bytes of fp8 seg per class
ROW_B = DET_B + SEG_B      # 4704 bytes per (n, c)

F32 = mybir.dt.float32
BF16 = mybir.dt.bfloat16
FP8 = mybir.dt.float8e4
NP_FP8 = ml_dtypes.float8_e4m3
U8 = mybir.dt.uint8

X_BUFS = 4
PSUM_BUFS = 4
WARMUP_MMS = 8

_program = None


def _build_program():
    nc = bacc.Bacc(
        "TRN2", target_bir_lowering=False, debug=False, num_devices=N_CORES
    )
    x_d = nc.dram_tensor("x", [NCH, NS, CPC, ROW_B], U8, kind="ExternalInput")
    # per-class edge-weighted diag masks: mask[g, c, s*G+g'] = E[c,s]*(g==g')
    mask_d = nc.dram_tensor(
        "mask", [G, C_DET, C_SEG * G], BF16, kind="ExternalInput"
    )
    numer_d = nc.dram_tensor("numer", [G, C_DET], F32, kind="ExternalOutput")
    dsum_d = nc.dram_tensor("dsum", [NS, C_DET], F32, kind="ExternalOutput")

    with tile.TileContext(nc) as tc:
        with (
            tc.tile_pool(name="x", bufs=X_BUFS) as x_pool,
            tc.tile_pool(name="res", bufs=1) as res_pool,
            tc.tile_pool(name="psum", bufs=PSUM_BUFS, space="PSUM") as psum_pool,
        ):
            # PE warmup: dense dummy matmuls (zeroed operands) to flip the
            # HAM clock gate to 2.4 GHz while the first input DMA lands.
            warm_t = res_pool.tile([NS, 512], FP8)
            nc.gpsimd.memset(warm_t[:], 0.0)
            warm_ps = psum_pool.tile([8, 512], F32)
            for _ in range(WARMUP_MMS):
                nc.tensor.matmul(
                    warm_ps[:], warm_t[:, :8], warm_t[:, :512],
                    start=True, stop=True,
                )

            mask_t = res_pool.tile([G, C_DET, C_SEG * G], BF16)
            nc.scalar.dma_start(out=mask_t[:], in_=mask_d[:])
            numer_t = res_pool.tile([G, C_DET], F32)
            dsum_t = res_pool.tile([NS, C_DET], F32)
            scratch = res_pool.tile([G, C_SEG * G], F32)
            act_scratch = res_pool.tile([NS, HW], BF16)

            for j in range(NCH):
                x_t = x_pool.tile([NS, CPC, ROW_B], U8)
                nc.sync.dma_start(out=x_t[:], in_=x_d[j])
                for cj in range(CPC):
                    c = j * CPC + cj
                    det_v = x_t[:, cj, 0:DET_B].bitcast(BF16)       # [NS, 784]
                    seg_v = x_t[:, cj, DET_B:ROW_B].bitcast(FP8).rearrange(
                        "p (s hw) -> p s hw", s=C_SEG
                    )                                               # [NS, 4, 784]
                    nc.scalar.activation(
                        out=act_scratch[:],
                        in_=det_v,
                        func=mybir.ActivationFunctionType.Copy,
                        accum_out=dsum_t[:, c : c + 1],
                    )
                    psum_t = psum_pool.tile([G, C_SEG * G], F32)
                    for k in range(KCH):
                        nc.tensor.matmul(
                            psum_t[:],
                            det_v[:, k * G : (k + 1) * G],
                            seg_v[:, :, k * G : (k + 1) * G],
                            start=(k == 0),
                            stop=(k == KCH - 1),
                        )
                    nc.vector.scalar_tensor_tensor(
                        out=scratch[:],
                        in0=psum_t[:],
                        scalar=0.0,
                        in1=mask_t[:, c, :],
                        op0=mybir.AluOpType.bypass,
                        op1=mybir.AluOpType.mult,
                        accum_out=numer_t[:, c : c + 1],
                    )
            nc.sync.dma_start(out=numer_d[:], in_=numer_t[:])
            nc.sync.dma_start(out=dsum_d[:], in_=dsum_t[:])

    nc.compile()
    return nc


def _get_program():
    global _program
    if _program is None:
        _program = _build_program()
    return _program


def _sr_fp8(v, rng):
    """Exact stochastic rounding to fp8e4m3: E[q(v)] = v.

    For non-negative v below fp8 max, the e4m3 bit patterns are monotone,
    so the two neighbors of v are byte-adjacent.
    """
    q0 = v.astype(NP_FP8)
    f0 = q0.astype(np.float32)
    b = q0.view(np.uint8)
    lo_b = np.where(f0 <= v, b, b - 1).astype(np.uint8)
    hi_b = lo_b + 1
    lo = lo_b.view(NP_FP8).astype(np.float32)
    hi = hi_b.view(NP_FP8).astype(np.float32)
    p = (v - lo) / (hi - lo)
    u = rng.random(v.shape, dtype=np.float32)
    out_b = np.where(u < p, hi_b, lo_b).astype(np.uint8)
    # exactly-representable values keep their encoding
    out_b = np.where(f0 == v, b, out_b)
    return out_b.view(NP_FP8)


def _pack_inputs(det_mask_probs, seg_mask_probs):
    """[N,8,28,28] f32 + [N,8,4,28,28] f32 -> [cores, C_DET, NS, ROW_B] u8."""
    det = np.asarray(det_mask_probs, dtype=np.float32).reshape(
        N_CORES, NS, C_DET, HW
    )
    seg = np.asarray(seg_mask_probs, dtype=np.float32).reshape(
        N_CORES, NS, C_DET, C_SEG * HW
    )
    rng = np.random.default_rng(12345)
    det_b = det.astype(ml_dtypes.bfloat16).view(np.uint8)   # [.., C_DET, 1568]
    seg_b = _sr_fp8(seg, rng).view(np.uint8)                # [.., C_DET, 3136]
    packed = np.concatenate([det_b, seg_b], axis=3)         # [8, NS, C_DET, 4704]
    packed = packed.reshape(N_CORES, NS, NCH, CPC, ROW_B)
    packed = packed.transpose(0, 2, 1, 3, 4)                # [8, NCH, NS, CPC, ROW_B]
    return np.ascontiguousarray(packed)


def _edge_mask(edge_i, edge_j):
    """mask[g, c, s*G+g'] = E[c,s] * (g == g')."""
    E = np.zeros((C_DET, C_SEG), dtype=np.float32)
    np.add.at(E, (np.asarray(edge_j), np.asarray(edge_i)), 1.0)
    eye = np.eye(G, dtype=np.float32)
    blk = np.einsum("cs,gh->gcsh", E, eye)      # [G, C_DET, C_SEG, G]
    return np.ascontiguousarray(
        blk.reshape(G, C_DET, C_SEG * G).astype(ml_dtypes.bfloat16)
    )                                           # E counts <= 256: exact in bf16


def _run_device(det_mask_probs, seg_mask_probs, edge_i, edge_j, trace=False):
    """Run the per-core reduction on all 8 cores; return (numer, denom, res)."""
    nc = _get_program()
    x = _pack_inputs(det_mask_probs, seg_mask_probs)
    mask = _edge_mask(edge_i, edge_j)

    in_maps = [{"x": x[r], "mask": mask} for r in range(N_CORES)]
    res = run_bass_kernel_spmd(nc, in_maps, list(range(N_CORES)), trace=trace)

    numer = np.zeros((C_DET,), dtype=np.float64)
    denom = np.zeros((C_DET,), dtype=np.float64)
    for r in range(N_CORES):
        numer += res.results[r]["numer"].sum(axis=0)
        denom += res.results[r]["dsum"].sum(axis=0)
    return numer, denom, res


def _finish(det_class_probs, numer, denom):
    w = numer / denom  # (C_DET,)
    probs = np.asarray(det_class_probs, dtype=np.float64) @ w  # (N_OBJ,)
    bce = (-np.clip(np.log(probs), -100.0, None)).mean()
    return np.asarray(bce, dtype=np.float32)


def kernel(det_class_probs, det_mask_probs, seg_mask_probs, edge_i, edge_j):
    numer, denom, _ = _run_device(
        det_mask_probs, seg_mask_probs, edge_i, edge_j, trace=False
    )
    return _finish(det_class_probs, numer, denom)
